# revision 1
# baseline (speedup 1.0000x reference)
"""TRN2 Bass kernel for nn_BasicAttention (B=8, S=2048, D=1024, fp32).

out[b] = concat([x[b], softmax(x[b] @ y[b].T) @ y[b]], axis=-1)

Sharding: batch b -> NeuronCore b (8 cores, data parallel, no collectives).

Per-core pipeline (e^T orientation, fp16 matmuls, fp32 logits/stats):
  - y16 = fp16(y) via casting SWDGE DMA;  yT via hardware DMA-transpose
  - MM1: eT[j,i] = sum_d yT[d,j] * xT[d,i]  (fp16 PE matmuls, fp32 PSUM)
  - per-row max: DVE max-tree + PE-transpose partition-reduce +
    ones-matmul broadcast; ai = exp(eT - rowmax) -> fp16 (ACT)
  - MM2: a[i,d] = sum_j ai[j,i] * y16[j,d]; denominator via ones column
  - a *= 1/den (DVE), store; x passes through DRAM->DRAM
"""
import sys

if '/opt/trn_rl_repo' not in sys.path:
    sys.path.insert(0, '/opt/trn_rl_repo')

import json
import numpy as np

import bass_rust
import concourse.bass as bass
import concourse.mybir as mybir
from concourse.tile import TileContext
from concourse.masks import make_identity

F32 = mybir.dt.float32
F16 = mybir.dt.float16

B = 8             # batches == cores
S = 2048          # sequence length (Sx == Sy)
D = 1024          # feature dim
JT = S // 128     # 16 j-tiles
KT = D // 128     # 8 d-tiles (contraction)
IBS = 512         # i-block size
NIB = S // IBS    # 4 i-blocks
ITB = IBS // 128  # 4 i-tiles per block
NDC = D // 512    # 2 d-chunks for MM2 output


def _legalize_waits(nc):
    """This toolchain's walrus accepts at most ONE sync-wait per
    instruction. Hoist extra waits onto single-wait NoOps inserted just
    before the offending instruction on the same engine."""
    d = json.loads(bass_rust.module_to_json_string(nc.m))
    nfix = 0
    for fn in d["functions"]:
        for bb in fn["blocks"]:
            new_insts = []
            for inst in bb["instructions"]:
                si = inst.get("sync_info")
                ow = si.get("on_wait", []) if si else []
                if len(ow) > 1:
                    for w in ow[:-1]:
                        nfix += 1
                        new_insts.append({
                            "engine": inst["engine"],
                            "ins": [], "outs": [],
                            "name": f"waitfix-{nfix}",
                            "opcode": "NoOp",
                            "sync_info": {"on_update": [], "on_wait": [w]},
                        })
                    si["on_wait"] = [ow[-1]]
                new_insts.append(inst)
            bb["instructions"] = new_insts
    nc.m = bass_rust.module_from_json_string(json.dumps(d))
    return nc


def build_attention_nc(reps=1):
    nc = bass.Bass(trn_type="TRN2", target_bir_lowering=False)
    x = nc.dram_tensor("x", [S, D], F32, kind="ExternalInput")
    y = nc.dram_tensor("y", [S, D], F32, kind="ExternalInput")
    out = nc.dram_tensor("out", [S, 2 * D], F32, kind="ExternalOutput")

    with TileContext(nc) as tc:
        with tc.tile_pool(name="persist", bufs=1) as persist, \
             tc.tile_pool(name="blk", bufs=1) as blk, \
             tc.tile_pool(name="small", bufs=2) as small, \
             tc.tile_pool(name="pe_ps", bufs=3, space="PSUM") as pe_ps, \
             tc.tile_pool(name="a_ps", bufs=2, space="PSUM") as a_ps, \
             tc.tile_pool(name="tp_ps", bufs=2, space="PSUM") as tp_ps, \
             tc.tile_pool(name="st_ps", bufs=1, space="PSUM") as st_ps:

            ident16 = persist.tile([128, 128], F16, tag="ident16")
            make_identity(nc, ident16[:])
            ident32 = persist.tile([128, 128], F32, tag="ident32")
            make_identity(nc, ident32[:])
            ones32 = persist.tile([1, 128], F16, tag="ones32")
            nc.vector.memset(ones32[:], 1.0)
            ones16 = persist.tile([128, 1], F16, tag="ones16")
            nc.vector.memset(ones16[:], 1.0)

            # ---- y: cast to fp16 (natural) and build yT via DMA transpose ----
            y16 = persist.tile([128, JT, D], F16, tag="y16")
            for jt in range(JT):
                nc.gpsimd.dma_start(out=y16[:, jt, :], in_=y[jt * 128:(jt + 1) * 128, :])
            yT = persist.tile([128, KT, S], F16, tag="yT")
            for jt in range(JT):
                for kt in range(KT):
                    tp = tp_ps.tile([128, 128], F16, tag="tp")
                    nc.tensor.transpose(tp[:], y16[:, jt, kt * 128:(kt + 1) * 128], ident16[:])
                    nc.scalar.copy(out=yT[:, kt, jt * 128:(jt + 1) * 128], in_=tp[:])

            for _rep in range(reps):
              for ib in range(NIB):
                  i0 = ib * IBS
                  # ---- x block: cast + transpose ----
                  x16 = blk.tile([128, ITB, D], F16, tag="x16", bufs=1)
                  for it in range(ITB):
                      nc.gpsimd.dma_start(
                          out=x16[:, it, :],
                          in_=x[i0 + it * 128:i0 + (it + 1) * 128, :])
                  xT = blk.tile([128, KT, IBS], F16, tag="xT", bufs=2)
                  for kt in range(KT):
                      for it in range(ITB):
                          tp = tp_ps.tile([128, 128], F16, tag="tp")
                          nc.tensor.transpose(tp[:], x16[:, it, kt * 128:(kt + 1) * 128], ident16[:])
                          nc.scalar.copy(out=xT[:, kt, it * 128:(it + 1) * 128], in_=tp[:])

                  # ---- MM1 + incremental max tree ----
                  eT = blk.tile([128, JT, IBS], F32, tag="eT", bufs=2)
                  m = blk.tile([128, IBS], F32, tag="m", bufs=2)
                  for jt in range(JT):
                      ps = pe_ps.tile([128, IBS], F32, tag="pe")
                      for kt in range(KT):
                          nc.tensor.matmul(
                              ps[:],
                              yT[:, kt, jt * 128:(jt + 1) * 128],
                              xT[:, kt, :],
                              start=(kt == 0), stop=(kt == KT - 1))
                      nc.scalar.copy(out=eT[:, jt, :], in_=ps[:])
                      if jt == 1:
                          nc.vector.tensor_max(m[:], eT[:, 0, :], eT[:, 1, :])
                      elif jt > 1:
                          nc.vector.tensor_max(m[:], m[:], eT[:, jt, :])

                  # ---- cross-partition row max -> broadcast tile ----
                  stat = st_ps.tile([128, 4, 128], F32, tag="stat")
                  rowmax_p = small.tile([128, 4], F32, tag="rowmax_p")
                  for q in range(4):
                      nc.tensor.transpose(stat[:, q, :], m[:, q * 128:(q + 1) * 128], ident32[:])
                      nc.vector.tensor_reduce(
                          out=rowmax_p[:, q:q + 1], in_=stat[:, q, :],
                          axis=mybir.AxisListType.X, op=mybir.AluOpType.max)
                  nc.tensor.transpose(stat[:4, 0, :], rowmax_p[:, :], ident32[:])
                  rm_sb = small.tile([4, 128], F16, tag="rm_sb")
                  nc.vector.tensor_copy(out=rm_sb[:], in_=stat[:4, 0, :])
                  rm_row = small.tile([1, IBS], F16, tag="rm_row")
                  nc.gpsimd.dma_start(out=rm_row[:1, :], in_=rm_sb[:, :])
                  bc_ps = st_ps.tile([128, IBS], F32, tag="stat")
                  nc.tensor.matmul(bc_ps[:], ones32[:1, :], rm_row[:1, :], start=True, stop=True)
                  bc_sb = small.tile([128, IBS], F32, tag="bc_sb")
                  nc.vector.tensor_copy(out=bc_sb[:], in_=bc_ps[:])

                  # ---- subtract + exp -> ai fp16 ----
                  ai = blk.tile([128, JT, IBS], F16, tag="ai", bufs=1)
                  for jt in range(JT):
                      nc.vector.tensor_sub(eT[:, jt, :], eT[:, jt, :], bc_sb[:])
                      nc.scalar.activation(
                          out=ai[:, jt, :], in_=eT[:, jt, :],
                          func=mybir.ActivationFunctionType.Exp)

                  # ---- MM2 + denominators + scale ----
                  aout = blk.tile([128, ITB, D], F32, tag="aout", bufs=1)
                  for it in range(ITB):
                      isl = slice(it * 128, (it + 1) * 128)
                      dps = st_ps.tile([128, 1], F32, tag="stat")
                      for jt in range(JT):
                          nc.tensor.matmul(
                              dps[:], ai[:, jt, isl], ones16[:, :1],
                              start=(jt == 0), stop=(jt == JT - 1))
                      den_sb = small.tile([128, 1], F32, tag="den_sb")
                      nc.vector.reciprocal(out=den_sb[:], in_=dps[:])
                      for c in range(NDC):
                          aps = a_ps.tile([128, 512], F32, tag="a")
                          for jt in range(JT):
                              nc.tensor.matmul(
                                  aps[:], ai[:, jt, isl], y16[:, jt, c * 512:(c + 1) * 512],
                                  start=(jt == 0), stop=(jt == JT - 1))
                          nc.vector.tensor_scalar_mul(
                              aout[:, it, c * 512:(c + 1) * 512], aps[:], den_sb[:])

                  # ---- store ----
                  for it in range(ITB):
                      r0 = i0 + it * 128
                      nc.gpsimd.dma_start(out=out[r0:r0 + 128, D:2 * D], in_=aout[:, it, :])
                      nc.gpsimd.dma_start(out=out[r0:r0 + 128, 0:D], in_=x[r0:r0 + 128, :])
    return nc


class _Runner:
    """Compile once; run with device-resident sharded inputs via PJRT."""

    def __init__(self, reps=1):
        import jax
        from jax.sharding import Mesh, PartitionSpec, NamedSharding
        from jax.experimental.shard_map import shard_map
        from concourse import bass2jax
        from concourse.bass2jax import _bass_exec_p, install_neuronx_cc_hook

        install_neuronx_cc_hook()
        nc = _legalize_waits(build_attention_nc(reps=reps))
        self.nc = nc
        partition_name = nc.partition_id_tensor.name if nc.partition_id_tensor else None
        in_names, out_names, out_avals = [], [], []
        zero_specs = []
        for alloc in nc.m.functions[0].allocations:
            if not isinstance(alloc, mybir.MemoryLocationSet):
                continue
            name = alloc.memorylocations[0].name
            if alloc.kind == "ExternalInput":
                if name != partition_name:
                    in_names.append(name)
            elif alloc.kind == "ExternalOutput":
                out_names.append(name)
                shape = tuple(alloc.tensor_shape)
                dtype = mybir.dt.np(alloc.dtype)
                out_avals.append(jax.core.ShapedArray(shape, dtype))
                zero_specs.append((shape, dtype))
        self.in_names, self.out_names, self.out_avals = in_names, out_names, out_avals
        n_params, n_outs = len(in_names), len(out_names)

        def _body(*args):
            operands = list(args)
            if partition_name is not None:
                operands.append(bass2jax.partition_id_tensor())
            outs = _bass_exec_p.bind(
                *operands,
                out_avals=tuple(out_avals),
                in_names=tuple(in_names + out_names
                               + ([partition_name] if partition_name else [])),
                out_names=tuple(out_names),
                lowering_input_output_aliases=(),
                sim_require_finite=True,
                sim_require_nnan=True,
                nc=nc,
            )
            return tuple(outs)

        devices = jax.devices()[:B]
        self.mesh = Mesh(np.asarray(devices), ("core",))
        in_specs = (PartitionSpec("core"),) * (n_params + n_outs)
        out_specs = (PartitionSpec("core"),) * n_outs
        donate = tuple(range(n_params, n_params + n_outs))
        self.sharded = jax.jit(
            shard_map(_body, mesh=self.mesh, in_specs=in_specs,
                      out_specs=out_specs, check_rep=False),
            donate_argnums=donate, keep_unused=True)
        self.sharding = NamedSharding(self.mesh, PartitionSpec("core"))
        import jax.numpy as jnp
        zshapes = [(B * s[0], *s[1:]) for s, _ in zero_specs]
        zdtypes = [dt for _, dt in zero_specs]
        self._mk_zeros = jax.jit(
            lambda: tuple(jnp.zeros(s, d) for s, d in zip(zshapes, zdtypes)),
            out_shardings=tuple(self.sharding for _ in zshapes))
        self._jax = jax

    def put_inputs(self, per_core_maps):
        concat = [np.concatenate([np.asarray(m[name]) for m in per_core_maps], axis=0)
                  for name in self.in_names]
        return [self._jax.device_put(a, self.sharding) for a in concat]

    def run_raw(self, in_dev):
        outs = self.sharded(*in_dev, *self._mk_zeros())
        self._jax.block_until_ready(outs)
        return outs

    def run(self, per_core_maps):
        outs = self.run_raw(self.put_inputs(per_core_maps))
        res = []
        for c in range(B):
            res.append({
                name: np.asarray(outs[i]).reshape(B, *self.out_avals[i].shape)[c]
                for i, name in enumerate(self.out_names)})
        return res


_RUNNER_CACHE = {}


def _get_runner(reps=1):
    if reps not in _RUNNER_CACHE:
        _RUNNER_CACHE[reps] = _Runner(reps=reps)
    return _RUNNER_CACHE[reps]


def kernel(x: np.ndarray, y: np.ndarray) -> np.ndarray:
    """Full-input entry point: x [8,2048,1024] f32, y [8,2048,1024] f32
    -> out [8,2048,2048] f32."""
    x = np.asarray(x, dtype=np.float32)
    y = np.asarray(y, dtype=np.float32)
    assert x.shape == (B, S, D) and y.shape == (B, S, D)
    r = _get_runner(reps=1)
    maps = [{"x": x[c], "y": y[c]} for c in range(B)]
    res = r.run(maps)
    return np.stack([res[c]["out"] for c in range(B)])

